# revision 1
# baseline (speedup 1.0000x reference)
"""Multi-head causal attention (B=4, S=2048, D=1024, H=16, dk=dv=64) on 8
Trainium2 NeuronCores.

Sharding: batch (4) x head-group (2) -> 8 cores. Each core computes, for its
batch b and its 8 heads, the partial output (concat_g @ WO_g)^T in [D, S]
layout. Host sums the two head-group partials per batch, transposes, adds bO.

Per-core kernel (single SPMD Bass program, per-core data):
  P1: Q^T, K^T = (X @ WQg)^T, (X @ WKg)^T  as [512, 2048]  (j on partitions)
  P2: Vbar[s, h, 0:64] = (X @ WVg)[s, 512], Vbar[s, h, 64] = 1.0 (ones col)
  P3: per (q-block 512, head pair, k-chunk 128):
      scoresT[k, q] = K^T-slice.T @ Q^T-slice   (contraction d=64; even/odd
      head of the pair at partition offsets 0/64 -> disjoint PE row groups)
      attnT = exp(0.125 * scoresT)  (no max subtraction: |scores|/8 small)
      causal: fully-masked (kc,qb) tiles skipped, fully-masked left columns
      excluded from matmul N-range, diagonal-crossing 128 cols * tril mask
      fv_aug[0:65, q] += Vbar-chunk.T @ attnT    (row 64 = softmax denom)
      norm: rc = 1/denom (DVE approx), rb = partition_broadcast(rc) (gpsimd),
      CT[dv, q] = fv * rb  (concat^T chunks in SBUF)
  P4: OUTT[m, q] = sum_c WOg-chunk.T @ CT-chunk   (WO stationary)

dtypes: X and W inputs in bf16 (proj matmuls bf16); Q/K/V/attn/CT tiles are
float32r (full f32 bits stored -- DVE/ACT write through f32 bitcast views --
PE rounds to ~tf32 internally).
"""

import sys

sys.path.insert(0, "/opt/trn_rl_repo")

import numpy as np
import ml_dtypes

import concourse.bass as bass
from concourse import bacc
import concourse.tile as tile
from concourse import mybir
from concourse.bass_utils import run_bass_kernel_spmd

F32 = mybir.dt.float32
F32R = mybir.dt.float32r
BF16 = mybir.dt.bfloat16
EXP = mybir.ActivationFunctionType.Exp
ADT = BF16   # dtype for Q/K/V/attn tiles (attention matmuls)

S = 2048          # sequence length
D = 1024          # model dim
HG = 8            # heads per core
DK = 64           # head dim
JG = HG * DK      # 512 = projected dim per core
CK = D // 128     # 8 contraction chunks for projections
NJT = JG // 128   # 4 j-tiles (head pairs)
NSB = S // 512    # 4 s-blocks of 512
NST = S // 128    # 16 s-tiles of 128
NQB = S // 512    # 4 q-blocks of 512

_NC_CACHE = {}


def build_nc(salt=""):
    nc = bacc.Bacc("TRN2", target_bir_lowering=False)

    XT_d = nc.declare_dram_parameter("XT", [D, S], BF16, isOutput=False)
    WQ_d = nc.declare_dram_parameter("WQ", [D, JG], BF16, isOutput=False)
    WK_d = nc.declare_dram_parameter("WK", [D, JG], BF16, isOutput=False)
    WV_d = nc.declare_dram_parameter("WV", [D, JG], BF16, isOutput=False)
    WO_d = nc.declare_dram_parameter("WO", [JG, D], BF16, isOutput=False)
    OUT_d = nc.declare_dram_parameter("OUTT", [D, S], F32, isOutput=True)

    # tril-inclusive mask in (k, q) orientation: mask[kr, qr] = 1 iff qr >= kr
    tri_np = (np.arange(128)[None, :] >= np.arange(128)[:, None]).astype(
        mybir.dt.np(ADT))
    TRI_d = nc.inline_tensor(tri_np, name="trimask" + salt)

    with tile.TileContext(nc) as tc:
        with (
            tc.tile_pool(name="persist", bufs=1) as pp,
            tc.tile_pool(name="psum", bufs=1, space="PSUM") as ps,
        ):
            tri = pp.tile([128, 128], ADT, tag="tri", name="tri")
            nc.sync.dma_start(tri[:], TRI_d[:])

            QT = [pp.tile([128, S], ADT, tag=f"qt{j}", name=f"qt{j}")
                  for j in range(NJT)]
            KT = [pp.tile([128, S], ADT, tag=f"kt{j}", name=f"kt{j}")
                  for j in range(NJT)]
            VB = [pp.tile([128, HG, DK + 1], ADT, tag=f"vb{s}", name=f"vb{s}")
                  for s in range(NST)]

            with tc.tile_pool(name="stage", bufs=1) as sp:
                XT = [sp.tile([128, S], BF16, tag=f"xt{c}", name=f"xt{c}")
                      for c in range(CK)]
                WQ = [sp.tile([128, JG], BF16, tag=f"wq{c}", name=f"wq{c}")
                      for c in range(CK)]
                WK = [sp.tile([128, JG], BF16, tag=f"wk{c}", name=f"wk{c}")
                      for c in range(CK)]
                WV = [sp.tile([128, JG], BF16, tag=f"wv{c}", name=f"wv{c}")
                      for c in range(CK)]
                for c in range(CK):
                    nc.sync.dma_start(XT[c][:], XT_d[128 * c : 128 * (c + 1), :])
                    nc.sync.dma_start(WQ[c][:], WQ_d[128 * c : 128 * (c + 1), :])
                    nc.sync.dma_start(WK[c][:], WK_d[128 * c : 128 * (c + 1), :])
                    nc.sync.dma_start(WV[c][:], WV_d[128 * c : 128 * (c + 1), :])

                onestg = sp.tile([128, HG], F32, tag="onestg", name="onestg")
                nc.gpsimd.memset(onestg[:], 1.0)

                # ---- P1: Q^T and K^T projections ([j, s] layout) ----
                for W, OT in ((WQ, QT), (WK, KT)):
                    for jt in range(NJT):
                        for sb in range(NSB):
                            pq = ps.tile([128, 512], F32, tag="acc", name="acc",
                                         bufs=2)
                            for c in range(CK):
                                nc.tensor.matmul(
                                    pq[:],
                                    W[c][:, 128 * jt : 128 * (jt + 1)],
                                    XT[c][:, 512 * sb : 512 * (sb + 1)],
                                    start=(c == 0),
                                    stop=(c == CK - 1),
                                )
                            nc.vector.tensor_copy(
                                OT[jt][:, 512 * sb : 512 * (sb + 1)], pq[:]
                            )

                # ---- P2: V projection, seq-major, with ones column ----
                for st in range(NST):
                    pv = ps.tile([128, 512], F32, tag="acc", name="acc", bufs=2)
                    for c in range(CK):
                        nc.tensor.matmul(
                            pv[:],
                            XT[c][:, 128 * st : 128 * (st + 1)],
                            WV[c][:],
                            start=(c == 0),
                            stop=(c == CK - 1),
                        )
                    nc.vector.tensor_copy(
                        VB[st][:, :, 0:DK],
                        pv[:].rearrange("p (h d) -> p h d", h=HG),
                    )
                    nc.vector.tensor_copy(
                        VB[st][:, :, DK : DK + 1],
                        onestg[:].rearrange("p (h u) -> p h u", u=1),
                    )

            with tc.tile_pool(name="late", bufs=1) as lp:
                WO = [lp.tile([128, D], BF16, tag=f"wo{c}", name=f"wo{c}")
                      for c in range(NJT)]
                for c in range(NJT):
                    nc.sync.dma_start(
                        WO[c][:], WO_d[128 * c : 128 * (c + 1), :]
                    )
                CT = [
                    [lp.tile([128, 512], BF16, tag=f"ct{qb}_{hp}",
                             name=f"ct{qb}_{hp}") for hp in range(NJT)]
                    for qb in range(NQB)
                ]

                coll = [lp.tile([HG, 512], F32, tag=f"coll{qb}",
                                name=f"coll{qb}") for qb in range(NQB)]

                # ---- P3: attention per (q-block, head-pair) ----
                for qb in range(NQB):
                    for hp in range(NJT):
                        nkc = 4 * qb + 4
                        fv = [ps.tile([DK + 1, 512], F32, tag=f"fv{p}",
                                      name=f"fv{p}") for p in (0, 1)]
                        for kc in range(nkc):
                            o = max(0, 128 * kc - 512 * qb)
                            sc = ps.tile([128, 1024], F32, tag="sc", name="sc",
                                         bufs=2)
                            for p in (0, 1):
                                pr = 64 * p
                                nc.tensor.matmul(
                                    sc[:, 512 * p + o : 512 * (p + 1)],
                                    KT[hp][pr : pr + 64,
                                           128 * kc : 128 * (kc + 1)],
                                    QT[hp][pr : pr + 64,
                                           512 * qb + o : 512 * (qb + 1)],
                                    start=True,
                                    stop=True,
                                )
                            at = lp.tile([128, 1024], ADT, tag="attnT",
                                         name="attnT", bufs=4)
                            if o == 0:
                                nc.scalar.activation(at[:], sc[:], EXP,
                                                     scale=0.125)
                            else:
                                for p in (0, 1):
                                    nc.scalar.activation(
                                        at[:, 512 * p + o : 512 * (p + 1)],
                                        sc[:, 512 * p + o : 512 * (p + 1)],
                                        EXP,
                                        scale=0.125,
                                    )
                            if kc >= 4 * qb:  # diagonal-crossing tile
                                for p in (0, 1):
                                    blk = at[:, 512 * p + o : 512 * p + o + 128]
                                    nc.vector.tensor_mul(blk, blk, tri[:])
                            for p in (0, 1):
                                h = 2 * hp + p
                                nc.tensor.matmul(
                                    fv[p][:, o:512],
                                    VB[kc][:, h, :],
                                    at[:, 512 * p + o : 512 * (p + 1)],
                                    start=(kc == 0),
                                    stop=(kc == nkc - 1),
                                )
                        for p in (0, 1):
                            nc.vector.tensor_copy(
                                CT[qb][hp][64 * p : 64 * (p + 1), :],
                                fv[p][0:64, :],
                            )
                            dr = lp.tile([1, 512], F32, tag="dr", name="dr",
                                         bufs=2)
                            nc.vector.tensor_copy(dr[:], fv[p][64:65, :])
                            nc.sync.dma_start(
                                coll[qb][2 * hp + p : 2 * hp + p + 1, :], dr[:]
                            )

                    # batch softmax denominators for this q-block:
                    # one wide reciprocal, then per-head broadcast + in-place
                    # CT scale, then this q-block's out-projection.
                    nc.vector.reciprocal(coll[qb][:], coll[qb][:])
                    for hp in range(NJT):
                        for p in (0, 1):
                            r = 2 * hp + p
                            rc = lp.tile([1, 512], F32, tag="rc", name="rc",
                                         bufs=4)
                            nc.sync.dma_start(rc[:], coll[qb][r : r + 1, :])
                            rb = lp.tile([128, 512], F32, tag="rb", name="rb",
                                         bufs=4)
                            nc.gpsimd.partition_broadcast(rb[:], rc[:])
                            ct_sl = CT[qb][hp][64 * p : 64 * (p + 1), :]
                            nc.vector.tensor_mul(
                                ct_sl, ct_sl, rb[64 * p : 64 * (p + 1), :]
                            )
                    for mt in range(D // 128):
                        po = ps.tile([128, 512], F32, tag="acc", name="acc",
                                     bufs=2)
                        for c in range(NJT):
                            nc.tensor.matmul(
                                po[:],
                                WO[c][:, 128 * mt : 128 * (mt + 1)],
                                CT[qb][c][:],
                                start=(c == 0),
                                stop=(c == NJT - 1),
                            )
                        og = lp.tile([128, 512], F32, tag="ostg", name="ostg",
                                     bufs=3)
                        nc.vector.tensor_copy(og[:], po[:])
                        nc.sync.dma_start(
                            OUT_d[128 * mt : 128 * (mt + 1),
                                  512 * qb : 512 * (qb + 1)],
                            og[:],
                        )
    nc.finalize()
    return nc


def _get_nc():
    if "nc" not in _NC_CACHE:
        _NC_CACHE["nc"] = build_nc()
    return _NC_CACHE["nc"]


def _make_in_maps(XKV, WQ, WK, WV, WO):
    bf = ml_dtypes.bfloat16
    in_maps = []
    for core in range(8):
        b, g = core // 2, core % 2
        sl = slice(512 * g, 512 * (g + 1))
        in_maps.append(
            {
                "XT": np.ascontiguousarray(XKV[b].T).astype(bf),
                "WQ": np.ascontiguousarray(WQ[:, sl]).astype(bf),
                "WK": np.ascontiguousarray(WK[:, sl]).astype(bf),
                "WV": np.ascontiguousarray(WV[:, sl]).astype(bf),
                "WO": np.ascontiguousarray(WO[sl, :]).astype(bf),
            }
        )
    return in_maps


def _combine(results, bO):
    out = np.empty((4, S, D), dtype=np.float32)
    for b in range(4):
        acc = results[2 * b]["OUTT"] + results[2 * b + 1]["OUTT"]
        out[b] = acc.T + bO[None, :]
    return out


def kernel(XKV, WQ, WK, WV, WO, bO):
    XKV = np.asarray(XKV, dtype=np.float32)
    nc = _get_nc()
    in_maps = _make_in_maps(XKV, np.asarray(WQ), np.asarray(WK), np.asarray(WV),
                            np.asarray(WO))
    res = run_bass_kernel_spmd(nc, in_maps, list(range(8)))
    return _combine(res.results, np.asarray(bO, dtype=np.float32))



# revision 6
# speedup vs baseline: 1.0300x; 1.0300x over previous
"""Multi-head causal attention (B=4, S=2048, D=1024, H=16, dk=dv=64) on 8
Trainium2 NeuronCores.

Sharding: batch (4) x head-group (2) -> 8 cores. Each core computes, for its
batch b and its 8 heads, the partial output (concat_g @ WO_g)^T in [D, S]
layout. Host sums the two head-group partials per batch, transposes, adds bO.

Per-core kernel (single SPMD Bass program, per-core data):
  P1: Q^T, K^T = (X @ WQg)^T, (X @ WKg)^T  as [512, 2048]  (j on partitions)
  P2: Vbar[s, h, 0:64] = (X @ WVg)[s, 512], Vbar[s, h, 64] = 1.0 (ones col)
  P3 (software-pipelined, skew 2): per (q-block 512, head pair, k-chunk 128):
      front(i): scoresT_p[k, q] = K^T-slice.T @ Q^T-slice per head p (two
      [128,512] PSUM tiles, K=64 row-quadrants run packed on the PE),
      at_p = exp(0.125 * scoresT_p) (scalar engine, per-p), tri-mask the
      diagonal-crossing 128 cols (DVE).
      back(i-2): fv_p[0:65, q] += Vbar-chunk.T @ at_p  (row 64 = denom)
      -> the fv matmuls trail the score matmuls by 2 iterations so the
      scalar-engine exp latency never stalls the PE.
      hp end: denom rows DMA'd PSUM->SBUF, reciprocal_approx_fast,
      gpsimd partition_broadcast, fused scale+cast into CT (bf16).
  P4 (lagged 3 iterations into the next head-pair's scores): OUTT[m, q] =
      sum_c WOg-chunk.T @ CT-chunk, DMA'd to DRAM directly from PSUM.

dtypes: inputs bf16; scores/fv accumulate f32 in PSUM; attn tiles bf16.
"""

import sys

sys.path.insert(0, "/opt/trn_rl_repo")

import numpy as np
import ml_dtypes

import concourse.bass as bass
from concourse import bacc
import concourse.tile as tile
from concourse import mybir
from concourse.bass_utils import run_bass_kernel_spmd

F32 = mybir.dt.float32
BF16 = mybir.dt.bfloat16
EXP = mybir.ActivationFunctionType.Exp
ADT = BF16   # dtype for Q/K/V/attn tiles (attention matmuls)

S = 2048          # sequence length
D = 1024          # model dim
HG = 8            # heads per core
DK = 64           # head dim
JG = HG * DK      # 512 = projected dim per core
CK = D // 128     # 8 contraction chunks for projections
NJT = JG // 128   # 4 j-tiles (head pairs)
NSB = S // 512    # 4 s-blocks of 512
NST = S // 128    # 16 s-tiles of 128
NQB = S // 512    # 4 q-blocks of 512

SKEW = 2          # fv trails scores by this many (qb,hp,kc) iterations
PLAG = 3          # out-projection trails the end of its q-block by this many

_NC_CACHE = {}


def build_nc(salt=""):
    nc = bacc.Bacc("TRN2", target_bir_lowering=False)

    XT_d = nc.declare_dram_parameter("XT", [D, S], BF16, isOutput=False)
    WQ_d = nc.declare_dram_parameter("WQ", [D, JG], BF16, isOutput=False)
    WK_d = nc.declare_dram_parameter("WK", [D, JG], BF16, isOutput=False)
    WV_d = nc.declare_dram_parameter("WV", [D, JG], BF16, isOutput=False)
    WO_d = nc.declare_dram_parameter("WO", [JG, D], BF16, isOutput=False)
    OUT_d = nc.declare_dram_parameter("OUTT", [D, S], F32, isOutput=True)

    # tril-inclusive mask in (k, q) orientation: mask[kr, qr] = 1 iff qr >= kr
    tri_np = (np.arange(128)[None, :] >= np.arange(128)[:, None]).astype(
        mybir.dt.np(ADT))
    TRI_d = nc.inline_tensor(tri_np, name="trimask" + salt)

    with tile.TileContext(nc) as tc:
        with (
            tc.tile_pool(name="persist", bufs=1) as pp,
            tc.tile_pool(name="psum", bufs=1, space="PSUM") as ps,
        ):
            tri = pp.tile([128, 128], ADT, tag="tri", name="tri")
            nc.sync.dma_start(tri[:], TRI_d[:])

            QT = [pp.tile([128, S], ADT, tag=f"qt{j}", name=f"qt{j}")
                  for j in range(NJT)]
            KT = [pp.tile([128, S], ADT, tag=f"kt{j}", name=f"kt{j}")
                  for j in range(NJT)]
            VB = [pp.tile([128, HG, DK + 1], ADT, tag=f"vb{s}", name=f"vb{s}")
                  for s in range(NST)]

            with tc.tile_pool(name="stage", bufs=1) as sp:
                XT = [sp.tile([128, S], BF16, tag=f"xt{c}", name=f"xt{c}")
                      for c in range(CK)]
                WQ = [sp.tile([128, JG], BF16, tag=f"wq{c}", name=f"wq{c}")
                      for c in range(CK)]
                WK = [sp.tile([128, JG], BF16, tag=f"wk{c}", name=f"wk{c}")
                      for c in range(CK)]
                WV = [sp.tile([128, JG], BF16, tag=f"wv{c}", name=f"wv{c}")
                      for c in range(CK)]
                # DMA in consumption order: P1-Q needs XT+WQ first.
                for c in range(CK):
                    nc.sync.dma_start(XT[c][:], XT_d[128 * c : 128 * (c + 1), :])
                    nc.sync.dma_start(WQ[c][:], WQ_d[128 * c : 128 * (c + 1), :])
                for c in range(CK):
                    nc.sync.dma_start(WK[c][:], WK_d[128 * c : 128 * (c + 1), :])
                for c in range(CK):
                    nc.sync.dma_start(WV[c][:], WV_d[128 * c : 128 * (c + 1), :])

                # ---- P1: Q^T and K^T projections ([j, s] layout) ----
                for W, OT in ((WQ, QT), (WK, KT)):
                    for jt in range(NJT):
                        for sb in range(NSB):
                            pq = ps.tile([128, 512], F32, tag="ps", name="ps",
                                         bufs=4)
                            for c in range(CK):
                                nc.tensor.matmul(
                                    pq[:],
                                    W[c][:, 128 * jt : 128 * (jt + 1)],
                                    XT[c][:, 512 * sb : 512 * (sb + 1)],
                                    start=(c == 0),
                                    stop=(c == CK - 1),
                                )
                            nc.vector.tensor_copy(
                                OT[jt][:, 512 * sb : 512 * (sb + 1)], pq[:]
                            )

                # ---- P2: V projection, seq-major, with ones column ----
                for st in range(NST):
                    pv = ps.tile([128, 512], F32, tag="ps", name="ps", bufs=4)
                    for c in range(CK):
                        nc.tensor.matmul(
                            pv[:],
                            XT[c][:, 128 * st : 128 * (st + 1)],
                            WV[c][:],
                            start=(c == 0),
                            stop=(c == CK - 1),
                        )
                    nc.vector.tensor_copy(
                        VB[st][:, :, 0:DK],
                        pv[:].rearrange("p (h d) -> p h d", h=HG),
                    )
                    nc.gpsimd.memset(VB[st][:, :, DK : DK + 1], 1.0)

            with tc.tile_pool(name="late", bufs=1) as lp:
                WO = [lp.tile([128, D], BF16, tag=f"wo{c}", name=f"wo{c}")
                      for c in range(NJT)]
                for c in range(NJT):
                    nc.sync.dma_start(
                        WO[c][:], WO_d[128 * c : 128 * (c + 1), :]
                    )
                CT = [
                    [lp.tile([128, 512], BF16, tag=f"ct{qb}_{hp}",
                             name=f"ct{qb}_{hp}") for hp in range(NJT)]
                    for qb in range(NQB)
                ]

                # ---- P3: software-pipelined attention ----
                iters = [(qb, hp, kc)
                         for qb in range(NQB)
                         for hp in range(NJT)
                         for kc in range(4 * qb + 4)]
                n_it = len(iters)
                at_tiles = {}   # i -> (at_p0, at_p1)
                fv_tiles = {}   # (qb, hp) -> (fv_p0, fv_p1)
                pending_p4 = [] # (emit_step, qb)

                def emit_front(i):
                    qb, hp, kc = iters[i]
                    o = max(0, 128 * kc - 512 * qb)
                    ats = []
                    for p in (0, 1):
                        pr = 64 * p
                        sc = ps.tile([128, 512], F32, tag="ps", name="ps",
                                     bufs=4)
                        nc.tensor.matmul(
                            sc[:, o:512],
                            KT[hp][pr : pr + 64, 128 * kc : 128 * (kc + 1)],
                            QT[hp][pr : pr + 64,
                                   512 * qb + o : 512 * (qb + 1)],
                            start=True,
                            stop=True,
                        )
                        at = lp.tile([128, 512], ADT, tag=f"at{p}",
                                     name=f"at{p}", bufs=4)
                        nc.scalar.activation(at[:, o:512], sc[:, o:512], EXP,
                                             scale=0.125)
                        if kc >= 4 * qb:  # diagonal-crossing tile
                            blk = at[:, o : o + 128]
                            nc.vector.tensor_mul(blk, blk, tri[:])
                        ats.append(at)
                    at_tiles[i] = ats

                def emit_back(i):
                    qb, hp, kc = iters[i]
                    nkc = 4 * qb + 4
                    o = max(0, 128 * kc - 512 * qb)
                    if kc == 0:
                        fv_tiles[(qb, hp)] = [
                            ps.tile([DK + 1, 512], F32, tag=f"fv{p}",
                                    name=f"fv{p}", bufs=2) for p in (0, 1)
                        ]
                    fv = fv_tiles[(qb, hp)]
                    ats = at_tiles.pop(i)
                    for p in (0, 1):
                        h = 2 * hp + p
                        nc.tensor.matmul(
                            fv[p][:, o:512],
                            VB[kc][:, h, :],
                            ats[p][:, o:512],
                            start=(kc == 0),
                            stop=(kc == nkc - 1),
                        )
                    if kc == nkc - 1:
                        # head-pair epilogue: denominators -> recip ->
                        # broadcast -> fused scale+cast into CT
                        for p in (0, 1):
                            dr = lp.tile([1, 512], F32, tag=f"dr{p}",
                                         name=f"dr{p}", bufs=2)
                            nc.vector.tensor_copy(dr[:], fv[p][64:65, :])
                            nc.vector.reciprocal_approx_fast(dr[:], dr[:])
                            rb = lp.tile([128, 512], F32, tag=f"rb{p}",
                                         name=f"rb{p}", bufs=2)
                            nc.gpsimd.partition_broadcast(rb[:], dr[:])
                            nc.vector.tensor_mul(
                                CT[qb][hp][64 * p : 64 * (p + 1), :],
                                fv[p][0:64, :],
                                rb[64 * p : 64 * (p + 1), :],
                            )
                        if hp == NJT - 1:
                            pending_p4.append((i + PLAG, qb))

                def emit_p4(qb):
                    for mt in range(D // 128):
                        po = ps.tile([128, 512], F32, tag="ps", name="ps",
                                     bufs=4)
                        for c in range(NJT):
                            nc.tensor.matmul(
                                po[:],
                                WO[c][:, 128 * mt : 128 * (mt + 1)],
                                CT[qb][c][:],
                                start=(c == 0),
                                stop=(c == NJT - 1),
                            )
                        og = lp.tile([128, 512], F32, tag="ostg", name="ostg",
                                     bufs=3)
                        if mt % 2 == 0:
                            nc.vector.tensor_copy(og[:], po[:])
                        else:
                            nc.scalar.activation(
                                og[:], po[:],
                                mybir.ActivationFunctionType.Copy)
                        nc.sync.dma_start(
                            OUT_d[128 * mt : 128 * (mt + 1),
                                  512 * qb : 512 * (qb + 1)],
                            og[:],
                        )

                for step in range(n_it + SKEW):
                    if step < n_it:
                        emit_front(step)
                    if step >= SKEW:
                        emit_back(step - SKEW)
                    while pending_p4 and pending_p4[0][0] <= step - SKEW:
                        emit_p4(pending_p4.pop(0)[1])
                while pending_p4:
                    emit_p4(pending_p4.pop(0)[1])
    nc.finalize()
    return nc


def _get_nc():
    if "nc" not in _NC_CACHE:
        _NC_CACHE["nc"] = build_nc()
    return _NC_CACHE["nc"]


def _make_in_maps(XKV, WQ, WK, WV, WO):
    bf = ml_dtypes.bfloat16
    in_maps = []
    for core in range(8):
        b, g = core // 2, core % 2
        sl = slice(512 * g, 512 * (g + 1))
        in_maps.append(
            {
                "XT": np.ascontiguousarray(XKV[b].T).astype(bf),
                "WQ": np.ascontiguousarray(WQ[:, sl]).astype(bf),
                "WK": np.ascontiguousarray(WK[:, sl]).astype(bf),
                "WV": np.ascontiguousarray(WV[:, sl]).astype(bf),
                "WO": np.ascontiguousarray(WO[sl, :]).astype(bf),
            }
        )
    return in_maps


def _combine(results, bO):
    out = np.empty((4, S, D), dtype=np.float32)
    for b in range(4):
        acc = results[2 * b]["OUTT"] + results[2 * b + 1]["OUTT"]
        out[b] = acc.T + bO[None, :]
    return out


def kernel(XKV, WQ, WK, WV, WO, bO):
    XKV = np.asarray(XKV, dtype=np.float32)
    nc = _get_nc()
    in_maps = _make_in_maps(XKV, np.asarray(WQ), np.asarray(WK), np.asarray(WV),
                            np.asarray(WO))
    res = run_bass_kernel_spmd(nc, in_maps, list(range(8)))
    return _combine(res.results, np.asarray(bO, dtype=np.float32))


# revision 9
# speedup vs baseline: 1.1298x; 1.0969x over previous
"""Multi-head causal attention (B=4, S=2048, D=1024, H=16, dk=dv=64) on 8
Trainium2 NeuronCores.

Sharding: batch (4) x head-group (2) -> 8 cores. Each core computes, for its
batch b and its 8 heads, the partial output (concat_g @ WO_g)^T in [D, S]
layout. Host sums the two head-group partials per batch, transposes, adds bO.

Per-core kernel (single SPMD Bass program, per-core data):
  P1: Q^T, K^T = (X @ WQg)^T, (X @ WKg)^T  as [512, 2048]  (j on partitions)
  P2: Vbar[s, h, 0:64] = (X @ WVg)[s, 512], Vbar[s, h, 64] = 1.0 (ones col)
  P3 (software-pipelined, skew 2, pair-grouped): per iteration
      (q-block 512, head pair, k-chunk 128):
      front(i): scoresT[k, q] for both heads of the pair into one wide
      [128,1024] PSUM tile (K=64 row-quadrants, adjacent so the PE co-issues
      them), one wide exp(0.125*scores) on the scalar engine, tri-mask of the
      diagonal-crossing cols on DVE.
      back(i-2): fv_p[0:65, q] += Vbar-chunk.T @ at_p  (row 64 = denom).
      Iterations are emitted in PAIRS ([sc(g) sc(g+1)] then [fv(g-2)
      fv(g-3)]) and every tensor matmul carries a nosync edge to the
      previous one, pinning the engine order: score and fv matmuls never
      alternate (PE tile-reconfig flush), and the exp latency is hidden by
      two iterations of skew.
      hp end: denom rows -> reciprocal_approx_fast -> gpsimd broadcast ->
      fused scale+cast into CT (bf16).
  P4 (lagged into the next q-block's scores): OUTT[m, q] = sum_c
      WOg-chunk.T @ CT-chunk, staged out via scalar/vector copies + DMA.

dtypes: inputs bf16; scores/fv accumulate f32 in PSUM; attn tiles bf16.
"""

import sys

sys.path.insert(0, "/opt/trn_rl_repo")

import numpy as np
import ml_dtypes

import concourse.bass as bass
from concourse import bacc
import concourse.tile as tile
from concourse import mybir
from concourse.bass_utils import run_bass_kernel_spmd

F32 = mybir.dt.float32
BF16 = mybir.dt.bfloat16
EXP = mybir.ActivationFunctionType.Exp
COPY = mybir.ActivationFunctionType.Copy
ADT = BF16   # dtype for Q/K/V/attn tiles (attention matmuls)

S = 2048          # sequence length
D = 1024          # model dim
HG = 8            # heads per core
DK = 64           # head dim
JG = HG * DK      # 512 = projected dim per core
CK = D // 128     # 8 contraction chunks for projections
NJT = JG // 128   # 4 j-tiles (head pairs)
NSB = S // 512    # 4 s-blocks of 512
NST = S // 128    # 16 s-tiles of 128
NQB = S // 512    # 4 q-blocks of 512

SKEW = 2          # fv trails scores by this many (qb,hp,kc) iterations
PLAG = 3          # out-projection trails the end of its q-block by this many

_NC_CACHE = {}


def build_nc(salt=""):
    nc = bacc.Bacc("TRN2", target_bir_lowering=False)

    XT_d = nc.declare_dram_parameter("XT", [D, S], BF16, isOutput=False)
    WQ_d = nc.declare_dram_parameter("WQ", [D, JG], BF16, isOutput=False)
    WK_d = nc.declare_dram_parameter("WK", [D, JG], BF16, isOutput=False)
    WV_d = nc.declare_dram_parameter("WV", [D, JG], BF16, isOutput=False)
    WO_d = nc.declare_dram_parameter("WO", [JG, D], BF16, isOutput=False)
    OUT_d = nc.declare_dram_parameter("OUTT", [D, S], F32, isOutput=True)

    # tril-inclusive mask in (k, q) orientation: mask[kr, qr] = 1 iff qr >= kr
    tri_np = (np.arange(128)[None, :] >= np.arange(128)[:, None]).astype(
        mybir.dt.np(ADT))
    TRI_d = nc.inline_tensor(tri_np, name="trimask" + salt)

    # Pin the tensor-engine order to emission order (see module docstring).
    import bass_rust as _br
    prev_mm = [None]

    def tmm(out, lhsT, rhs, **kw):
        mm = nc.tensor.matmul(out, lhsT, rhs, **kw)
        if prev_mm[0] is not None:
            deps = _br.InstructionNameOrderedSet()
            deps.add(prev_mm[0])
            mm.ins.add_nosync_dependencies_from(deps)
        prev_mm[0] = mm.ins.name
        return mm

    with tile.TileContext(nc) as tc:
        with (
            tc.tile_pool(name="persist", bufs=1) as pp,
            tc.tile_pool(name="psum", bufs=1, space="PSUM") as ps,
        ):
            tri = pp.tile([128, 128], ADT, tag="tri", name="tri")
            nc.sync.dma_start(tri[:], TRI_d[:])

            QT = [pp.tile([128, S], ADT, tag=f"qt{j}", name=f"qt{j}")
                  for j in range(NJT)]
            KT = [pp.tile([128, S], ADT, tag=f"kt{j}", name=f"kt{j}")
                  for j in range(NJT)]
            VB = [pp.tile([128, HG, DK + 1], ADT, tag=f"vb{s}", name=f"vb{s}")
                  for s in range(NST)]

            with tc.tile_pool(name="stage", bufs=1) as sp:
                XT = [sp.tile([128, S], BF16, tag=f"xt{c}", name=f"xt{c}")
                      for c in range(CK)]
                WQ = [sp.tile([128, JG], BF16, tag=f"wq{c}", name=f"wq{c}")
                      for c in range(CK)]
                WK = [sp.tile([128, JG], BF16, tag=f"wk{c}", name=f"wk{c}")
                      for c in range(CK)]
                WV = [sp.tile([128, JG], BF16, tag=f"wv{c}", name=f"wv{c}")
                      for c in range(CK)]
                # DMA in consumption order: P1-Q needs XT+WQ first.
                for c in range(CK):
                    nc.sync.dma_start(XT[c][:], XT_d[128 * c : 128 * (c + 1), :])
                    nc.sync.dma_start(WQ[c][:], WQ_d[128 * c : 128 * (c + 1), :])
                for c in range(CK):
                    nc.sync.dma_start(WK[c][:], WK_d[128 * c : 128 * (c + 1), :])
                for c in range(CK):
                    nc.sync.dma_start(WV[c][:], WV_d[128 * c : 128 * (c + 1), :])

                # ---- P1: Q^T and K^T projections ([j, s] layout) ----
                # Wide PSUM tiles: two 512-col s-blocks per tile, one wide
                # PSUM->SBUF cast per pair.
                for W, OT in ((WQ, QT), (WK, KT)):
                    for jt in range(NJT):
                        for sb in (0, 2):
                            pq = ps.tile([128, 1024], F32, tag="scw",
                                         name="scw", bufs=2)
                            for half in (0, 1):
                                for c in range(CK):
                                    tmm(
                                        pq[:, 512 * half : 512 * (half + 1)],
                                        W[c][:, 128 * jt : 128 * (jt + 1)],
                                        XT[c][:, 512 * (sb + half)
                                              : 512 * (sb + half + 1)],
                                        start=(c == 0),
                                        stop=(c == CK - 1),
                                    )
                            nc.vector.tensor_copy(
                                OT[jt][:, 512 * sb : 512 * (sb + 2)], pq[:]
                            )

                # ---- P2: V projection, seq-major, with ones column ----
                for st2 in range(0, NST, 2):
                    pv = ps.tile([128, 1024], F32, tag="scw", name="scw",
                                 bufs=2)
                    for half in (0, 1):
                        st = st2 + half
                        for c in range(CK):
                            tmm(
                                pv[:, 512 * half : 512 * (half + 1)],
                                XT[c][:, 128 * st : 128 * (st + 1)],
                                WV[c][:],
                                start=(c == 0),
                                stop=(c == CK - 1),
                            )
                    for half in (0, 1):
                        st = st2 + half
                        nc.vector.tensor_copy(
                            VB[st][:, :, 0:DK],
                            pv[:, 512 * half : 512 * (half + 1)].rearrange(
                                "p (h d) -> p h d", h=HG),
                        )
                        nc.gpsimd.memset(VB[st][:, :, DK : DK + 1], 1.0)

            with tc.tile_pool(name="late", bufs=1) as lp:
                WO = [lp.tile([128, D], BF16, tag=f"wo{c}", name=f"wo{c}")
                      for c in range(NJT)]
                for c in range(NJT):
                    nc.sync.dma_start(
                        WO[c][:], WO_d[128 * c : 128 * (c + 1), :]
                    )
                CT = [
                    [lp.tile([128, 512], BF16, tag=f"ct{qb}_{hp}",
                             name=f"ct{qb}_{hp}") for hp in range(NJT)]
                    for qb in range(NQB)
                ]

                # ---- P3: software-pipelined attention ----
                iters = [(qb, hp, kc)
                         for qb in range(NQB)
                         for hp in range(NJT)
                         for kc in range(4 * qb + 4)]
                n_it = len(iters)
                at_tiles = {}   # i -> wide at tile
                fv_tiles = {}   # (qb, hp) -> (fv_p0, fv_p1)
                pending_p4 = [] # (emit_step, qb)

                def emit_front(i):
                    qb, hp, kc = iters[i]
                    o = max(0, 128 * kc - 512 * qb)
                    sc = ps.tile([128, 1024], F32, tag="scw", name="scw",
                                 bufs=2)
                    for p in (0, 1):
                        pr = 64 * p
                        tmm(
                            sc[:, 512 * p + o : 512 * (p + 1)],
                            KT[hp][pr : pr + 64, 128 * kc : 128 * (kc + 1)],
                            QT[hp][pr : pr + 64,
                                   512 * qb + o : 512 * (qb + 1)],
                            start=True,
                            stop=True,
                        )
                    at = lp.tile([128, 1024], ADT, tag="at", name="at",
                                 bufs=4)
                    if o == 0:
                        nc.scalar.activation(at[:], sc[:], EXP, scale=0.125)
                    else:
                        for p in (0, 1):
                            nc.scalar.activation(
                                at[:, 512 * p + o : 512 * (p + 1)],
                                sc[:, 512 * p + o : 512 * (p + 1)],
                                EXP,
                                scale=0.125,
                            )
                    if kc >= 4 * qb:  # diagonal-crossing tile
                        for p in (0, 1):
                            blk = at[:, 512 * p + o : 512 * p + o + 128]
                            nc.vector.tensor_mul(blk, blk, tri[:])
                    at_tiles[i] = at

                def emit_back(i):
                    qb, hp, kc = iters[i]
                    nkc = 4 * qb + 4
                    o = max(0, 128 * kc - 512 * qb)
                    if kc == 0:
                        fv_tiles[(qb, hp)] = [
                            ps.tile([DK + 1, 512], F32, tag=f"fv{p}",
                                    name=f"fv{p}", bufs=2) for p in (0, 1)
                        ]
                    fv = fv_tiles[(qb, hp)]
                    at = at_tiles.pop(i)
                    for p in (0, 1):
                        h = 2 * hp + p
                        tmm(
                            fv[p][:, o:512],
                            VB[kc][:, h, :],
                            at[:, 512 * p + o : 512 * (p + 1)],
                            start=(kc == 0),
                            stop=(kc == nkc - 1),
                        )
                    if kc == nkc - 1:
                        # head-pair epilogue: denominators -> recip ->
                        # broadcast -> fused scale+cast into CT
                        for p in (0, 1):
                            dr = lp.tile([1, 512], F32, tag=f"dr{p}",
                                         name=f"dr{p}", bufs=2)
                            nc.vector.tensor_copy(dr[:], fv[p][64:65, :])
                            nc.vector.reciprocal_approx_fast(dr[:], dr[:])
                            rb = lp.tile([128, 512], F32, tag=f"rb{p}",
                                         name=f"rb{p}", bufs=2)
                            nc.gpsimd.partition_broadcast(rb[:], dr[:])
                            nc.vector.tensor_mul(
                                CT[qb][hp][64 * p : 64 * (p + 1), :],
                                fv[p][0:64, :],
                                rb[64 * p : 64 * (p + 1), :],
                            )
                        if hp == NJT - 1:
                            pending_p4.append((i + PLAG, qb))

                def emit_p4(qb):
                    for mtp in range(D // 256):
                        po = ps.tile([128, 1024], F32, tag="scw", name="scw",
                                     bufs=2)
                        for half in (0, 1):
                            mt = 2 * mtp + half
                            for c in range(NJT):
                                tmm(
                                    po[:, 512 * half : 512 * (half + 1)],
                                    WO[c][:, 128 * mt : 128 * (mt + 1)],
                                    CT[qb][c][:],
                                    start=(c == 0),
                                    stop=(c == NJT - 1),
                                )
                        for half in (0, 1):
                            mt = 2 * mtp + half
                            og = lp.tile([128, 512], F32, tag="ostg",
                                         name="ostg", bufs=3)
                            src = po[:, 512 * half : 512 * (half + 1)]
                            if mt % 2 == 0:
                                nc.vector.tensor_copy(og[:], src)
                            else:
                                nc.scalar.activation(og[:], src, COPY)
                            nc.sync.dma_start(
                                OUT_d[128 * mt : 128 * (mt + 1),
                                      512 * qb : 512 * (qb + 1)],
                                og[:],
                            )

                for g in range(0, n_it + SKEW, 2):
                    for j in (g, g + 1):
                        if j < n_it:
                            emit_front(j)
                    for j in (g - SKEW, g + 1 - SKEW):
                        if 0 <= j < n_it:
                            emit_back(j)
                    while pending_p4 and pending_p4[0][0] <= g + 1 - SKEW:
                        emit_p4(pending_p4.pop(0)[1])
                while pending_p4:
                    emit_p4(pending_p4.pop(0)[1])
    nc.finalize()
    return nc


def _get_nc():
    if "nc" not in _NC_CACHE:
        _NC_CACHE["nc"] = build_nc()
    return _NC_CACHE["nc"]


def _make_in_maps(XKV, WQ, WK, WV, WO):
    bf = ml_dtypes.bfloat16
    in_maps = []
    for core in range(8):
        b, g = core // 2, core % 2
        sl = slice(512 * g, 512 * (g + 1))
        in_maps.append(
            {
                "XT": np.ascontiguousarray(XKV[b].T).astype(bf),
                "WQ": np.ascontiguousarray(WQ[:, sl]).astype(bf),
                "WK": np.ascontiguousarray(WK[:, sl]).astype(bf),
                "WV": np.ascontiguousarray(WV[:, sl]).astype(bf),
                "WO": np.ascontiguousarray(WO[sl, :]).astype(bf),
            }
        )
    return in_maps


def _combine(results, bO):
    out = np.empty((4, S, D), dtype=np.float32)
    for b in range(4):
        acc = results[2 * b]["OUTT"] + results[2 * b + 1]["OUTT"]
        out[b] = acc.T + bO[None, :]
    return out


def kernel(XKV, WQ, WK, WV, WO, bO):
    XKV = np.asarray(XKV, dtype=np.float32)
    nc = _get_nc()
    in_maps = _make_in_maps(XKV, np.asarray(WQ), np.asarray(WK), np.asarray(WV),
                            np.asarray(WO))
    res = run_bass_kernel_spmd(nc, in_maps, list(range(8)))
    return _combine(res.results, np.asarray(bO, dtype=np.float32))
